# revision 8
# baseline (speedup 1.0000x reference)
"""MoE (top-1 routed + 1 shared expert) Trainium2 kernel.

Strategy (v1, dense expert-parallel):
  - 8 NeuronCores, expert n on core n (expert 7 is the always-on shared expert).
  - Each core: f32 router (logits = x @ Wg, argmax among bias-eligible experts),
    then computes its expert over ALL tokens in bf16, scales each token's output
    by its combine gate (0 or ~1 for routed experts; 1 for the shared expert),
    writes a full [M, D] f32 partial.
  - Host combine: sum of the 8 partials.

Shapes are hardcoded for the problem instance:
  B=2, S=1024, D=1024, H=1024, N=8 experts, top-1 routed + shared expert 7.
"""

import numpy as np
import ml_dtypes

import concourse.bass as bass
import concourse.mybir as mybir
from concourse import bacc
from concourse.tile import TileContext
from concourse.bass_utils import run_bass_kernel_spmd

B, S, D, H, N = 2, 1024, 1024, 1024, 8
M = B * S            # 2048 tokens
NT = M // 128        # 16 token tiles
ND = D // 128        # 8 contraction chunks
NPAIR = H // 128     # 8 (g,u) pairs of 128-wide h tiles
TOKC = 512           # token chunk for expert matmuls
NTC = M // TOKC      # 4 token chunks

f32 = mybir.dt.float32
bf16 = mybir.dt.bfloat16
AF = mybir.ActivationFunctionType
OP = mybir.AluOpType

_built = None


def _build():
    nc = bacc.Bacc("TRN2", target_bir_lowering=False, debug=False)

    xT = nc.dram_tensor("xT", [D, M], f32, kind="ExternalInput")
    wg = nc.dram_tensor("wg", [D, N], f32, kind="ExternalInput")
    w1r = nc.dram_tensor("w1r", [D, 2 * H], bf16, kind="ExternalInput")
    w2 = nc.dram_tensor("w2", [H, D], bf16, kind="ExternalInput")
    bias128 = nc.dram_tensor("bias128", [1, 128], f32, kind="ExternalInput")
    sel128 = nc.dram_tensor("sel128", [1, 128], f32, kind="ExternalInput")
    gaff = nc.dram_tensor("gaff", [1, 2], f32, kind="ExternalInput")
    y = nc.dram_tensor("y", [M, D], f32, kind="ExternalOutput")
    dbg_tsel = nc.dram_tensor("dbg_tsel", [128, NT * N], f32, kind="ExternalOutput")
    dbg_gate = nc.dram_tensor("dbg_gate", [128, NT], f32, kind="ExternalOutput")

    xT_t = xT[:, :].rearrange("(c p) m -> p c m", p=128)      # [128, ND, M]
    wg_t = wg[:, :].rearrange("(c p) n -> p c n", p=128)      # [128, ND, N]
    w1_t = w1r[:, :].rearrange("(c p) h -> p c h", p=128)     # [128, ND, 2H]
    w2_t = w2[:, :].rearrange("(c p) d -> p c d", p=128)      # [128, NPAIR, D]
    y_t = y[:, :].rearrange("(t p) d -> p t d", p=128)        # [128, NT, D]

    with TileContext(nc) as tc:
        with (
            tc.tile_pool(name="consts", bufs=1) as consts,
            tc.tile_pool(name="weights", bufs=1) as wpool,
            tc.tile_pool(name="xbf", bufs=1) as xbfpool,
            tc.tile_pool(name="router", bufs=1) as router,
        ):
            # ---- constants / weights ----
            wg_sb = consts.tile([128, ND, N], f32)
            nc.sync.dma_start(wg_sb[:], wg_t)
            bias_row = consts.tile([1, 128], f32)
            nc.sync.dma_start(bias_row[:], bias128[:, :])
            sel_row = consts.tile([1, 128], f32)
            nc.sync.dma_start(sel_row[:], sel128[:, :])
            gaff_row = consts.tile([1, 2], f32)
            nc.sync.dma_start(gaff_row[:], gaff[:, :])
            ones_col = consts.tile([1, 128], f32)
            nc.vector.memset(ones_col[:], 1.0)

            w1_sb = wpool.tile([128, ND, 2 * H], bf16)
            nc.sync.dma_start(w1_sb[:], w1_t)
            w2_sb = wpool.tile([128, NPAIR, D], bf16)
            nc.sync.dma_start(w2_sb[:], w2_t)

            with tc.tile_pool(name="ps_ph1", bufs=2, space="PSUM") as ps_ph1:
                # broadcast rows across partitions via ones-matmul (K=1)
                bias_ps = ps_ph1.tile([128, 128], f32, tag="bc")
                nc.tensor.matmul(bias_ps[:], ones_col[:], bias_row[:])
                bias_rep = consts.tile([128, 128], f32)
                nc.vector.tensor_copy(bias_rep[:], bias_ps[:])

                sel_ps = ps_ph1.tile([128, 128], f32, tag="bc")
                nc.tensor.matmul(sel_ps[:], ones_col[:], sel_row[:])
                sel_rep = consts.tile([128, 128], f32)
                nc.vector.tensor_copy(sel_rep[:], sel_ps[:])

                ga_ps = ps_ph1.tile([128, 2], f32, tag="bc")
                nc.tensor.matmul(ga_ps[:], ones_col[:], gaff_row[:])
                ga_rep = consts.tile([128, 2], f32)
                nc.vector.tensor_copy(ga_rep[:], ga_ps[:])

                # ---- phase 1: load xT (f32, resident), cast to bf16 ----
                xbf = xbfpool.tile([128, ND, M], bf16)
                xtf = xbfpool.tile([128, ND, M], f32)
                for c in range(ND):
                    nc.sync.dma_start(xtf[:, c, :], xT_t[:, c, :])
                    nc.vector.tensor_copy(xbf[:, c, :], xtf[:, c, :])

                # router matmuls: one accumulation group at a time, ping-pong
                logits_sb = router.tile([128, NT, N], f32)
                for tt in range(NT):
                    lg_ps = ps_ph1.tile([128, N], f32, tag="lg")
                    for c in range(ND):
                        nc.tensor.matmul(
                            lg_ps[:],
                            xtf[:, c, tt * 128:(tt + 1) * 128],
                            wg_sb[:, c, :],
                            start=(c == 0),
                            stop=(c == ND - 1),
                        )
                    nc.vector.tensor_copy(logits_sb[:, tt, :], lg_ps[:])

                # ---- phase 2: router epilogue -> per-token gate [128, NT] ----
                scores = router.tile([128, NT, N], f32)
                nc.scalar.activation(scores[:].rearrange("p t n -> p (t n)"),
                                     logits_sb[:].rearrange("p t n -> p (t n)"),
                                     AF.Sigmoid)
                tsel = router.tile([128, NT, N], f32)
                nc.vector.tensor_tensor(
                    out=tsel[:].rearrange("p t n -> p (t n)"),
                    in0=logits_sb[:].rearrange("p t n -> p (t n)"),
                    in1=bias_rep[:],
                    op=OP.add,
                )
                tmax = router.tile([128, NT], f32)
                nc.vector.tensor_reduce(tmax[:], tsel[:], axis=mybir.AxisListType.X,
                                        op=OP.max)
                oh = router.tile([128, NT, N], f32)
                for tt in range(NT):
                    nc.vector.tensor_scalar(
                        out=oh[:, tt, :], in0=tsel[:, tt, :],
                        scalar1=tmax[:, tt:tt + 1], scalar2=None, op0=OP.is_equal,
                    )
                ohs = router.tile([128, NT, N], f32)
                nc.vector.tensor_tensor(
                    out=ohs[:].rearrange("p t n -> p (t n)"),
                    in0=oh[:].rearrange("p t n -> p (t n)"),
                    in1=scores[:].rearrange("p t n -> p (t n)"), op=OP.mult)
                s_sel = router.tile([128, NT], f32)
                nc.vector.tensor_reduce(s_sel[:], ohs[:], axis=mybir.AxisListType.X,
                                        op=OP.add)
                den = router.tile([128, NT], f32)
                nc.vector.tensor_scalar(out=den[:], in0=s_sel[:], scalar1=1e-9,
                                        scalar2=None, op0=OP.add)
                den_r = router.tile([128, NT], f32)
                nc.vector.reciprocal(den_r[:], den[:])
                wgt = router.tile([128, NT], f32)
                nc.vector.tensor_tensor(out=wgt[:], in0=s_sel[:], in1=den_r[:],
                                        op=OP.mult)
                ohm = router.tile([128, NT, N], f32)
                nc.vector.tensor_tensor(
                    out=ohm[:].rearrange("p t n -> p (t n)"),
                    in0=oh[:].rearrange("p t n -> p (t n)"),
                    in1=sel_rep[:], op=OP.mult)
                msk = router.tile([128, NT], f32)
                nc.vector.tensor_reduce(msk[:], ohm[:], axis=mybir.AxisListType.X,
                                        op=OP.add)
                g0 = router.tile([128, NT], f32)
                nc.vector.tensor_tensor(out=g0[:], in0=msk[:], in1=wgt[:], op=OP.mult)
                gate = router.tile([128, NT], f32)
                # gate = g0 * ga + gb   (cores 0..6: (1,0); core 7: (0,1))
                nc.vector.tensor_scalar(
                    out=gate[:], in0=g0[:],
                    scalar1=ga_rep[:, 0:1], scalar2=ga_rep[:, 1:2],
                    op0=OP.mult, op1=OP.add,
                )

                nc.sync.dma_start(dbg_tsel[:, :], tsel[:].rearrange("p t n -> p (t n)"))
                nc.sync.dma_start(dbg_gate[:, :], gate[:])

            # ---- phase 3: expert compute over token chunks ----
            with (
                tc.tile_pool(name="hbuf", bufs=2) as hbuf,
                tc.tile_pool(name="ybuf", bufs=2) as ybuf,
                tc.tile_pool(name="ps_gu", bufs=2, space="PSUM") as ps_gu,
                tc.tile_pool(name="ps_y", bufs=2, space="PSUM") as ps_y,
            ):
                for tk in range(NTC):
                    tok = slice(tk * TOKC, (tk + 1) * TOKC)
                    h_sb = hbuf.tile([128, NPAIR, TOKC], bf16, tag="h")
                    for pair in range(NPAIR):
                        g_ps = ps_gu.tile([128, TOKC], f32, tag="g")
                        u_ps = ps_gu.tile([128, TOKC], f32, tag="u")
                        for c in range(ND):
                            nc.tensor.matmul(
                                g_ps[:],
                                w1_sb[:, c, (2 * pair) * 128:(2 * pair + 1) * 128],
                                xbf[:, c, tok],
                                start=(c == 0), stop=(c == ND - 1),
                            )
                            nc.tensor.matmul(
                                u_ps[:],
                                w1_sb[:, c, (2 * pair + 1) * 128:(2 * pair + 2) * 128],
                                xbf[:, c, tok],
                                start=(c == 0), stop=(c == ND - 1),
                            )
                        sg = hbuf.tile([128, TOKC], f32, tag="sg")
                        nc.scalar.activation(sg[:], g_ps[:], AF.Silu)
                        nc.vector.tensor_tensor(out=h_sb[:, pair, :], in0=sg[:],
                                                in1=u_ps[:], op=OP.mult)

                    for tl in range(TOKC // 128):
                        gtt = tk * (TOKC // 128) + tl
                        y_sb = ybuf.tile([128, D], f32, tag="yout")
                        for dh in range(2):
                            y_ps = ps_y.tile([128, 512], f32, tag="yps")
                            for hc in range(NPAIR):
                                nc.tensor.matmul(
                                    y_ps[:],
                                    h_sb[:, hc, tl * 128:(tl + 1) * 128],
                                    w2_sb[:, hc, dh * 512:(dh + 1) * 512],
                                    start=(hc == 0), stop=(hc == NPAIR - 1),
                                )
                            nc.vector.tensor_scalar(
                                out=y_sb[:, dh * 512:(dh + 1) * 512], in0=y_ps[:],
                                scalar1=gate[:, gtt:gtt + 1], scalar2=None,
                                op0=OP.mult,
                            )
                        nc.sync.dma_start(y_t[:, gtt, :], y_sb[:])

    nc.compile()
    return nc


def _get_built():
    global _built
    if _built is None:
        _built = _build()
    return _built


def kernel(x_BSD, Wg_DN, Wl1_ND2H, Wl2_NHD, biases_N):
    x = np.asarray(x_BSD, dtype=np.float32).reshape(M, D)
    Wg = np.ascontiguousarray(np.asarray(Wg_DN, dtype=np.float32))
    W1 = np.asarray(Wl1_ND2H, dtype=np.float32)
    W2 = np.asarray(Wl2_NHD, dtype=np.float32)
    biases = np.asarray(biases_N, dtype=np.float32)

    xT = np.ascontiguousarray(x.T)                       # [D, M] f32

    # interleave W1 columns into (g_i, u_i) 128-col pairs
    w1r = np.empty((N, D, 2 * H), dtype=np.float32)
    for i in range(NPAIR):
        w1r[:, :, (2 * i) * 128:(2 * i + 1) * 128] = \
            W1[:, :, i * 128:(i + 1) * 128]
        w1r[:, :, (2 * i + 1) * 128:(2 * i + 2) * 128] = \
            W1[:, :, H + i * 128:H + (i + 1) * 128]

    bias_c = np.maximum(biases, -1e30)                   # clamp -inf
    bias128 = np.tile(bias_c, NT)[None, :].astype(np.float32)   # [1, 128]

    nc = _get_built()

    in_maps = []
    for core in range(N):
        sel = np.zeros(N, dtype=np.float32)
        if core < N - 1:
            sel[core] = 1.0
        sel128 = np.tile(sel, NT)[None, :]
        ga = np.array([[1.0, 0.0]] if core < N - 1 else [[0.0, 1.0]],
                      dtype=np.float32)
        in_maps.append({
            "xT": xT,
            "wg": Wg,
            "w1r": w1r[core].astype(ml_dtypes.bfloat16),
            "w2": np.ascontiguousarray(W2[core]).astype(ml_dtypes.bfloat16),
            "bias128": bias128,
            "sel128": np.ascontiguousarray(sel128),
            "gaff": ga,
        })

    global _last_in_maps
    _last_in_maps = in_maps

    res = run_bass_kernel_spmd(nc, in_maps, core_ids=list(range(N)))
    global _last_res
    _last_res = res
    out = np.zeros((M, D), dtype=np.float32)
    for core in range(N):
        out += res.results[core]["y"]
    return out.reshape(B, S, D)


# revision 10
# speedup vs baseline: 329.7415x; 329.7415x over previous
"""MoE (top-1 routed + 1 shared expert) Trainium2 kernel.

Strategy (v1, dense expert-parallel):
  - 8 NeuronCores, expert n on core n (expert 7 is the always-on shared expert).
  - Each core: f32 router (logits = x @ Wg, argmax among bias-eligible experts),
    then computes its expert over ALL tokens in bf16, scales each token's output
    by its combine gate (0 or ~1 for routed experts; 1 for the shared expert),
    writes a full [M, D] f32 partial.
  - Host combine: sum of the 8 partials.

Shapes are hardcoded for the problem instance:
  B=2, S=1024, D=1024, H=1024, N=8 experts, top-1 routed + shared expert 7.
"""

import numpy as np
import ml_dtypes

import concourse.bass as bass
import concourse.mybir as mybir
from concourse import bacc
from concourse.tile import TileContext
from concourse.bass_utils import run_bass_kernel_spmd

B, S, D, H, N = 2, 1024, 1024, 1024, 8
M = B * S            # 2048 tokens
NT = M // 128        # 16 token tiles
ND = D // 128        # 8 contraction chunks
NPAIR = H // 128     # 8 (g,u) pairs of 128-wide h tiles
TOKC = 512           # token chunk for expert matmuls
NTC = M // TOKC      # 4 token chunks

f32 = mybir.dt.float32
bf16 = mybir.dt.bfloat16
AF = mybir.ActivationFunctionType
OP = mybir.AluOpType

_built = None


def _build(loop_n=None):
    nc = bacc.Bacc("TRN2", target_bir_lowering=False, debug=False)

    xT = nc.dram_tensor("xT", [D, M], f32, kind="ExternalInput")
    wg = nc.dram_tensor("wg", [D, N], f32, kind="ExternalInput")
    w1r = nc.dram_tensor("w1r", [D, 2 * H], bf16, kind="ExternalInput")
    w2 = nc.dram_tensor("w2", [H, D], bf16, kind="ExternalInput")
    bias128 = nc.dram_tensor("bias128", [1, 128], f32, kind="ExternalInput")
    sel128 = nc.dram_tensor("sel128", [1, 128], f32, kind="ExternalInput")
    gaff = nc.dram_tensor("gaff", [1, 2], f32, kind="ExternalInput")
    y = nc.dram_tensor("y", [M, D], f32, kind="ExternalOutput")
    dbg_tsel = nc.dram_tensor("dbg_tsel", [128, NT * N], f32, kind="ExternalOutput")
    dbg_gate = nc.dram_tensor("dbg_gate", [128, NT], f32, kind="ExternalOutput")

    xT_t = xT[:, :].rearrange("(c p) m -> p c m", p=128)      # [128, ND, M]
    wg_t = wg[:, :].rearrange("(c p) n -> p c n", p=128)      # [128, ND, N]
    w1_t = w1r[:, :].rearrange("(c p) h -> p c h", p=128)     # [128, ND, 2H]
    w2_t = w2[:, :].rearrange("(c p) d -> p c d", p=128)      # [128, NPAIR, D]
    y_t = y[:, :].rearrange("(t p) d -> p t d", p=128)        # [128, NT, D]

    import contextlib

    with TileContext(nc) as tc:
        loop_ctx = tc.For_i(0, loop_n, 1) if loop_n else contextlib.nullcontext()
        with (
            loop_ctx,
            tc.tile_pool(name="consts", bufs=1) as consts,
            tc.tile_pool(name="weights", bufs=1) as wpool,
            tc.tile_pool(name="xbf", bufs=1) as xbfpool,
            tc.tile_pool(name="router", bufs=1) as router,
        ):
            # ---- constants / weights ----
            wg_sb = consts.tile([128, ND, N], f32)
            nc.sync.dma_start(wg_sb[:], wg_t)
            bias_row = consts.tile([1, 128], f32)
            nc.sync.dma_start(bias_row[:], bias128[:, :])
            sel_row = consts.tile([1, 128], f32)
            nc.sync.dma_start(sel_row[:], sel128[:, :])
            gaff_row = consts.tile([1, 2], f32)
            nc.sync.dma_start(gaff_row[:], gaff[:, :])
            ones_col = consts.tile([1, 128], f32)
            nc.vector.memset(ones_col[:], 1.0)

            w1_sb = wpool.tile([128, ND, 2 * H], bf16)
            nc.sync.dma_start(w1_sb[:], w1_t)
            w2_sb = wpool.tile([128, NPAIR, D], bf16)
            nc.sync.dma_start(w2_sb[:], w2_t)

            with tc.tile_pool(name="ps_ph1", bufs=2, space="PSUM") as ps_ph1:
                # broadcast rows across partitions via ones-matmul (K=1)
                bias_ps = ps_ph1.tile([128, 128], f32, tag="bc")
                nc.tensor.matmul(bias_ps[:], ones_col[:], bias_row[:])
                bias_rep = consts.tile([128, 128], f32)
                nc.vector.tensor_copy(bias_rep[:], bias_ps[:])

                sel_ps = ps_ph1.tile([128, 128], f32, tag="bc")
                nc.tensor.matmul(sel_ps[:], ones_col[:], sel_row[:])
                sel_rep = consts.tile([128, 128], f32)
                nc.vector.tensor_copy(sel_rep[:], sel_ps[:])

                ga_ps = ps_ph1.tile([128, 2], f32, tag="bc")
                nc.tensor.matmul(ga_ps[:], ones_col[:], gaff_row[:])
                ga_rep = consts.tile([128, 2], f32)
                nc.vector.tensor_copy(ga_rep[:], ga_ps[:])

                # ---- phase 1: load xT (f32, resident), cast to bf16 ----
                xbf = xbfpool.tile([128, ND, M], bf16)
                xtf = xbfpool.tile([128, ND, M], f32)
                for c in range(ND):
                    nc.sync.dma_start(xtf[:, c, :], xT_t[:, c, :])
                    nc.vector.tensor_copy(xbf[:, c, :], xtf[:, c, :])

                # router matmuls: one accumulation group at a time, ping-pong
                logits_sb = router.tile([128, NT, N], f32)
                for tt in range(NT):
                    lg_ps = ps_ph1.tile([128, N], f32, tag="lg")
                    for c in range(ND):
                        nc.tensor.matmul(
                            lg_ps[:],
                            xtf[:, c, tt * 128:(tt + 1) * 128],
                            wg_sb[:, c, :],
                            start=(c == 0),
                            stop=(c == ND - 1),
                        )
                    nc.vector.tensor_copy(logits_sb[:, tt, :], lg_ps[:])

                # ---- phase 2: router epilogue -> per-token gate [128, NT] ----
                scores = router.tile([128, NT, N], f32)
                nc.scalar.activation(scores[:].rearrange("p t n -> p (t n)"),
                                     logits_sb[:].rearrange("p t n -> p (t n)"),
                                     AF.Sigmoid)
                tsel = router.tile([128, NT, N], f32)
                nc.vector.tensor_tensor(
                    out=tsel[:].rearrange("p t n -> p (t n)"),
                    in0=logits_sb[:].rearrange("p t n -> p (t n)"),
                    in1=bias_rep[:],
                    op=OP.add,
                )
                tmax = router.tile([128, NT], f32)
                nc.vector.tensor_reduce(tmax[:], tsel[:], axis=mybir.AxisListType.X,
                                        op=OP.max)
                oh = router.tile([128, NT, N], f32)
                for tt in range(NT):
                    nc.vector.tensor_scalar(
                        out=oh[:, tt, :], in0=tsel[:, tt, :],
                        scalar1=tmax[:, tt:tt + 1], scalar2=None, op0=OP.is_equal,
                    )
                ohs = router.tile([128, NT, N], f32)
                nc.vector.tensor_tensor(
                    out=ohs[:].rearrange("p t n -> p (t n)"),
                    in0=oh[:].rearrange("p t n -> p (t n)"),
                    in1=scores[:].rearrange("p t n -> p (t n)"), op=OP.mult)
                s_sel = router.tile([128, NT], f32)
                nc.vector.tensor_reduce(s_sel[:], ohs[:], axis=mybir.AxisListType.X,
                                        op=OP.add)
                den = router.tile([128, NT], f32)
                nc.vector.tensor_scalar(out=den[:], in0=s_sel[:], scalar1=1e-9,
                                        scalar2=None, op0=OP.add)
                den_r = router.tile([128, NT], f32)
                nc.vector.reciprocal(den_r[:], den[:])
                wgt = router.tile([128, NT], f32)
                nc.vector.tensor_tensor(out=wgt[:], in0=s_sel[:], in1=den_r[:],
                                        op=OP.mult)
                ohm = router.tile([128, NT, N], f32)
                nc.vector.tensor_tensor(
                    out=ohm[:].rearrange("p t n -> p (t n)"),
                    in0=oh[:].rearrange("p t n -> p (t n)"),
                    in1=sel_rep[:], op=OP.mult)
                msk = router.tile([128, NT], f32)
                nc.vector.tensor_reduce(msk[:], ohm[:], axis=mybir.AxisListType.X,
                                        op=OP.add)
                g0 = router.tile([128, NT], f32)
                nc.vector.tensor_tensor(out=g0[:], in0=msk[:], in1=wgt[:], op=OP.mult)
                gate = router.tile([128, NT], f32)
                # gate = g0 * ga + gb   (cores 0..6: (1,0); core 7: (0,1))
                nc.vector.tensor_scalar(
                    out=gate[:], in0=g0[:],
                    scalar1=ga_rep[:, 0:1], scalar2=ga_rep[:, 1:2],
                    op0=OP.mult, op1=OP.add,
                )

                nc.sync.dma_start(dbg_tsel[:, :], tsel[:].rearrange("p t n -> p (t n)"))
                nc.sync.dma_start(dbg_gate[:, :], gate[:])

            # ---- phase 3: expert compute over token chunks ----
            with (
                tc.tile_pool(name="hbuf", bufs=2) as hbuf,
                tc.tile_pool(name="ybuf", bufs=2) as ybuf,
                tc.tile_pool(name="ps_gu", bufs=2, space="PSUM") as ps_gu,
                tc.tile_pool(name="ps_y", bufs=2, space="PSUM") as ps_y,
            ):
                for tk in range(NTC):
                    tok = slice(tk * TOKC, (tk + 1) * TOKC)
                    h_sb = hbuf.tile([128, NPAIR, TOKC], bf16, tag="h")
                    for pair in range(NPAIR):
                        g_ps = ps_gu.tile([128, TOKC], f32, tag="g")
                        u_ps = ps_gu.tile([128, TOKC], f32, tag="u")
                        for c in range(ND):
                            nc.tensor.matmul(
                                g_ps[:],
                                w1_sb[:, c, (2 * pair) * 128:(2 * pair + 1) * 128],
                                xbf[:, c, tok],
                                start=(c == 0), stop=(c == ND - 1),
                            )
                            nc.tensor.matmul(
                                u_ps[:],
                                w1_sb[:, c, (2 * pair + 1) * 128:(2 * pair + 2) * 128],
                                xbf[:, c, tok],
                                start=(c == 0), stop=(c == ND - 1),
                            )
                        sg = hbuf.tile([128, TOKC], f32, tag="sg")
                        nc.scalar.activation(sg[:], g_ps[:], AF.Silu)
                        nc.vector.tensor_tensor(out=h_sb[:, pair, :], in0=sg[:],
                                                in1=u_ps[:], op=OP.mult)

                    for tl in range(TOKC // 128):
                        gtt = tk * (TOKC // 128) + tl
                        y_sb = ybuf.tile([128, D], f32, tag="yout")
                        for dh in range(2):
                            y_ps = ps_y.tile([128, 512], f32, tag="yps")
                            for hc in range(NPAIR):
                                nc.tensor.matmul(
                                    y_ps[:],
                                    h_sb[:, hc, tl * 128:(tl + 1) * 128],
                                    w2_sb[:, hc, dh * 512:(dh + 1) * 512],
                                    start=(hc == 0), stop=(hc == NPAIR - 1),
                                )
                            nc.vector.tensor_scalar(
                                out=y_sb[:, dh * 512:(dh + 1) * 512], in0=y_ps[:],
                                scalar1=gate[:, gtt:gtt + 1], scalar2=None,
                                op0=OP.mult,
                            )
                        nc.sync.dma_start(y_t[:, gtt, :], y_sb[:])

    nc.compile()
    return nc


def _get_built():
    global _built
    if _built is None:
        _built = _build()
    return _built


_built_loop = {}


def _get_built_loop(n):
    if n not in _built_loop:
        _built_loop[n] = _build(loop_n=n)
    return _built_loop[n]


def kernel(x_BSD, Wg_DN, Wl1_ND2H, Wl2_NHD, biases_N):
    x = np.asarray(x_BSD, dtype=np.float32).reshape(M, D)
    Wg = np.ascontiguousarray(np.asarray(Wg_DN, dtype=np.float32))
    W1 = np.asarray(Wl1_ND2H, dtype=np.float32)
    W2 = np.asarray(Wl2_NHD, dtype=np.float32)
    biases = np.asarray(biases_N, dtype=np.float32)

    xT = np.ascontiguousarray(x.T)                       # [D, M] f32

    # interleave W1 columns into (g_i, u_i) 128-col pairs
    w1r = np.empty((N, D, 2 * H), dtype=np.float32)
    for i in range(NPAIR):
        w1r[:, :, (2 * i) * 128:(2 * i + 1) * 128] = \
            W1[:, :, i * 128:(i + 1) * 128]
        w1r[:, :, (2 * i + 1) * 128:(2 * i + 2) * 128] = \
            W1[:, :, H + i * 128:H + (i + 1) * 128]

    bias_c = np.maximum(biases, -1e30)                   # clamp -inf
    bias128 = np.tile(bias_c, NT)[None, :].astype(np.float32)   # [1, 128]

    nc = _get_built()

    in_maps = []
    for core in range(N):
        sel = np.zeros(N, dtype=np.float32)
        if core < N - 1:
            sel[core] = 1.0
        sel128 = np.tile(sel, NT)[None, :]
        ga = np.array([[1.0, 0.0]] if core < N - 1 else [[0.0, 1.0]],
                      dtype=np.float32)
        in_maps.append({
            "xT": xT,
            "wg": Wg,
            "w1r": w1r[core].astype(ml_dtypes.bfloat16),
            "w2": np.ascontiguousarray(W2[core]).astype(ml_dtypes.bfloat16),
            "bias128": bias128,
            "sel128": np.ascontiguousarray(sel128),
            "gaff": ga,
        })

    global _last_in_maps
    _last_in_maps = in_maps

    res = run_bass_kernel_spmd(nc, in_maps, core_ids=list(range(N)))
    global _last_res
    _last_res = res
    out = np.zeros((M, D), dtype=np.float32)
    for core in range(N):
        out += res.results[core]["y"]
    return out.reshape(B, S, D)


# revision 15
# speedup vs baseline: 999.0083x; 3.0297x over previous
"""MoE (top-1 routed + 1 shared expert) Trainium2 kernel.

Strategy (v2, sparse expert-parallel with on-device dispatch):
  - 8 NeuronCores. Core n (n<7) owns routed expert n; every core also computes
    a 256-token slice of the shared expert (expert 7).
  - Each core: f32 router over all 2048 tokens (logits = x @ Wg, top-1 among
    bias-eligible experts), on-device stream-compaction (gpsimd sparse_gather)
    of the tokens routed to this core's expert, capacity C=384, then a
    gather+transpose DMA pulls just those token rows (bf16) and the expert
    runs dense on the compacted [C] batch. Shared expert runs on a host-sliced
    256-token column block. Outputs are compact: [C, D] routed rows + token
    ids + count + [256, D] shared rows.
  - Host combine: scatter-add of the disjoint routed rows + slice-add of the
    shared rows.

Shapes hardcoded: B=2, S=1024, D=1024, H=1024, N=8, top-1 routed + shared.
"""

import numpy as np
import ml_dtypes

import concourse.bass as bass
import concourse.mybir as mybir
from concourse import bacc
from concourse.tile import TileContext
from concourse.bass_utils import run_bass_kernel_spmd

B, S, D, H, N = 2, 1024, 1024, 1024, 8
M = B * S            # 2048 tokens
NT = M // 128        # 16 token tiles
ND = D // 128        # 8 contraction chunks
NPAIR = H // 128     # 8 (g,u) pairs of 128-wide h tiles
CAP = 384            # routed token capacity per expert
NCT = CAP // 128     # 3 capacity tiles
CW = CAP // 16       # 24 wrapped columns
SSH = M // 8         # 256 shared tokens per core
NST = SSH // 128     # 2 shared token tiles

f32 = mybir.dt.float32
bf16 = mybir.dt.bfloat16
i16 = mybir.dt.int16
u32 = mybir.dt.uint32
AF = mybir.ActivationFunctionType
OP = mybir.AluOpType
AX = mybir.AxisListType

_built = None


def _build(loop_n=None):
    import contextlib

    nc = bacc.Bacc("TRN2", target_bir_lowering=False, debug=False)

    xT = nc.dram_tensor("xT", [D, M], f32, kind="ExternalInput")
    xbf_dram = nc.dram_tensor("xbf", [M, D], bf16, kind="ExternalInput")
    xshT = nc.dram_tensor("xshT", [D, SSH], f32, kind="ExternalInput")
    wg = nc.dram_tensor("wg", [D, N], f32, kind="ExternalInput")
    w1m = nc.dram_tensor("w1m", [D, 2 * H], bf16, kind="ExternalInput")
    w2m = nc.dram_tensor("w2m", [H, D], bf16, kind="ExternalInput")
    w1s = nc.dram_tensor("w1s", [D, 2 * H], bf16, kind="ExternalInput")
    w2s = nc.dram_tensor("w2s", [H, D], bf16, kind="ExternalInput")
    bias128 = nc.dram_tensor("bias128", [1, 128], f32, kind="ExternalInput")
    sel128 = nc.dram_tensor("sel128", [1, 128], f32, kind="ExternalInput")

    y_rt = nc.dram_tensor("y_rt", [CAP, D], f32, kind="ExternalOutput")
    y_sh = nc.dram_tensor("y_sh", [SSH, D], f32, kind="ExternalOutput")
    ids_out = nc.dram_tensor("ids_out", [16, CW], f32, kind="ExternalOutput")
    dbg_xg = nc.dram_tensor("dbg_xg", [128, ND * CAP], bf16, kind="ExternalOutput")
    dbg_gslot = nc.dram_tensor("dbg_gslot", [128, NCT], f32, kind="ExternalOutput")
    cnt_out = nc.dram_tensor("cnt_out", [1, 1], f32, kind="ExternalOutput")

    xT_t = xT[:, :].rearrange("(c p) m -> p c m", p=128)       # [128, ND, M]
    xshT_t = xshT[:, :].rearrange("(c p) m -> p c m", p=128)   # [128, ND, SSH]
    wg_t = wg[:, :].rearrange("(c p) n -> p c n", p=128)
    w1m_t = w1m[:, :].rearrange("(c p) h -> p c h", p=128)
    w2m_t = w2m[:, :].rearrange("(c p) d -> p c d", p=128)
    w1s_t = w1s[:, :].rearrange("(c p) h -> p c h", p=128)
    w2s_t = w2s[:, :].rearrange("(c p) d -> p c d", p=128)
    yrt_t = y_rt[:, :].rearrange("(t p) d -> p t d", p=128)    # [128, NCT, D]
    ysh_t = y_sh[:, :].rearrange("(t p) d -> p t d", p=128)    # [128, NST, D]

    with TileContext(nc) as tc:
        loop_ctx = tc.For_i(0, loop_n, 1) if loop_n else contextlib.nullcontext()
        with (
            loop_ctx,
            tc.tile_pool(name="consts", bufs=1) as consts,
            tc.tile_pool(name="weights", bufs=1) as wpool,
            tc.tile_pool(name="xpool", bufs=1) as xpool,
            tc.tile_pool(name="router", bufs=1) as router,
            tc.tile_pool(name="dram", bufs=1, space="DRAM") as dpool,
        ):
            # ---- constants / weights ----
            wg_sb = consts.tile([128, ND, N], f32)
            nc.sync.dma_start(wg_sb[:], wg_t)
            bias_row = consts.tile([1, 128], f32)
            nc.sync.dma_start(bias_row[:], bias128[:, :])
            sel_row = consts.tile([1, 128], f32)
            nc.sync.dma_start(sel_row[:], sel128[:, :])
            ones_col = consts.tile([1, 128], f32)
            nc.vector.memset(ones_col[:], 1.0)

            w1s_sb = wpool.tile([128, ND, 2 * H], bf16, tag="w1s")
            nc.sync.dma_start(w1s_sb[:], w1s_t)
            w2s_sb = wpool.tile([128, NPAIR, D], bf16, tag="w2s")
            nc.sync.dma_start(w2s_sb[:], w2s_t)
            w1m_sb = wpool.tile([128, ND, 2 * H], bf16, tag="w1m")
            nc.sync.dma_start(w1m_sb[:], w1m_t)
            w2m_sb = wpool.tile([128, NPAIR, D], bf16, tag="w2m")
            nc.sync.dma_start(w2m_sb[:], w2m_t)

            # ---- load xT (f32, resident) and shared-slice xT ----
            xtf = xpool.tile([128, ND, M], f32)
            for c in range(ND):
                nc.sync.dma_start(xtf[:, c, :], xT_t[:, c, :])
            xsh_b = xpool.tile([128, ND, SSH], bf16)
            with tc.tile_pool(name="xshf", bufs=1) as xshf_pool:
                xsh_f = xshf_pool.tile([128, ND, SSH], f32)
                nc.sync.dma_start(xsh_f[:], xshT_t)
                nc.vector.tensor_copy(
                    xsh_b[:].rearrange("p c m -> p (c m)"),
                    xsh_f[:].rearrange("p c m -> p (c m)"))

            with tc.tile_pool(name="ps_ph1", bufs=2, space="PSUM") as ps_ph1:
                # broadcast rows across partitions via ones-matmul (K=1)
                bias_ps = ps_ph1.tile([128, 128], f32, tag="bc")
                nc.tensor.matmul(bias_ps[:], ones_col[:], bias_row[:])
                bias_rep = consts.tile([128, 128], f32)
                nc.vector.tensor_copy(bias_rep[:], bias_ps[:])

                sel_ps = ps_ph1.tile([128, 128], f32, tag="bc")
                nc.tensor.matmul(sel_ps[:], ones_col[:], sel_row[:])
                sel_rep = consts.tile([128, 128], f32)
                nc.vector.tensor_copy(sel_rep[:], sel_ps[:])

                # router matmuls: one accumulation group at a time, ping-pong
                logits_sb = router.tile([128, NT, N], f32)
                for tt in range(NT):
                    lg_ps = ps_ph1.tile([128, N], f32, tag="lg")
                    for c in range(ND):
                        nc.tensor.matmul(
                            lg_ps[:],
                            xtf[:, c, tt * 128:(tt + 1) * 128],
                            wg_sb[:, c, :],
                            start=(c == 0), stop=(c == ND - 1),
                        )
                    nc.vector.tensor_copy(logits_sb[:, tt, :], lg_ps[:])

                # ---- router epilogue: mask + gate in [tok%128, tok//128] ----
                scores = router.tile([128, NT, N], f32)
                nc.scalar.activation(scores[:].rearrange("p t n -> p (t n)"),
                                     logits_sb[:].rearrange("p t n -> p (t n)"),
                                     AF.Sigmoid)
                tsel = router.tile([128, NT, N], f32)
                nc.vector.tensor_tensor(
                    out=tsel[:].rearrange("p t n -> p (t n)"),
                    in0=logits_sb[:].rearrange("p t n -> p (t n)"),
                    in1=bias_rep[:], op=OP.add)
                tmax = router.tile([128, NT], f32)
                nc.vector.tensor_reduce(tmax[:], tsel[:], axis=AX.X, op=OP.max)
                oh = router.tile([128, NT, N], f32)
                for tt in range(NT):
                    nc.vector.tensor_scalar(
                        out=oh[:, tt, :], in0=tsel[:, tt, :],
                        scalar1=tmax[:, tt:tt + 1], scalar2=None,
                        op0=OP.is_equal)
                ohs = router.tile([128, NT, N], f32)
                nc.vector.tensor_tensor(
                    out=ohs[:].rearrange("p t n -> p (t n)"),
                    in0=oh[:].rearrange("p t n -> p (t n)"),
                    in1=scores[:].rearrange("p t n -> p (t n)"), op=OP.mult)
                s_sel = router.tile([128, NT], f32)
                nc.vector.tensor_reduce(s_sel[:], ohs[:], axis=AX.X, op=OP.add)
                den = router.tile([128, NT], f32)
                nc.vector.tensor_scalar(out=den[:], in0=s_sel[:], scalar1=1e-9,
                                        scalar2=None, op0=OP.add)
                den_r = router.tile([128, NT], f32)
                nc.vector.reciprocal(den_r[:], den[:])
                wgt = router.tile([128, NT], f32)
                nc.vector.tensor_tensor(out=wgt[:], in0=s_sel[:], in1=den_r[:],
                                        op=OP.mult)
                ohm = router.tile([128, NT, N], f32)
                nc.vector.tensor_tensor(
                    out=ohm[:].rearrange("p t n -> p (t n)"),
                    in0=oh[:].rearrange("p t n -> p (t n)"),
                    in1=sel_rep[:], op=OP.mult)
                msk = router.tile([128, NT], f32)
                nc.vector.tensor_reduce(msk[:], ohm[:], axis=AX.X, op=OP.add)
                gate = router.tile([128, NT], f32)
                nc.vector.tensor_tensor(out=gate[:], in0=msk[:], in1=wgt[:],
                                        op=OP.mult)

                # ---- compaction: ids/gates of selected tokens ----
                iota_t = router.tile([128, NT], f32)
                nc.gpsimd.iota(iota_t[:], pattern=[[128, NT]], base=1,
                               channel_multiplier=1,
                               allow_small_or_imprecise_dtypes=True)
                vids = router.tile([128, NT], f32)
                # msk*(m+1) - 1  ->  m if selected else -1
                nc.vector.tensor_tensor(out=vids[:], in0=msk[:], in1=iota_t[:],
                                        op=OP.mult)
                nc.vector.tensor_scalar(out=vids[:], in0=vids[:], scalar1=-1.0,
                                        scalar2=None, op0=OP.add)
                gp1 = router.tile([128, NT], f32)
                nc.vector.tensor_scalar(out=gp1[:], in0=gate[:], scalar1=1.0,
                                        scalar2=None, op0=OP.add)
                vg = router.tile([128, NT], f32)
                nc.vector.tensor_tensor(out=vg[:], in0=msk[:], in1=gp1[:],
                                        op=OP.mult)
                nc.vector.tensor_scalar(out=vg[:], in0=vg[:], scalar1=-1.0,
                                        scalar2=None, op0=OP.add)

                # bounce [128, NT] (tok-partition) -> [16, 128] (wrapped)
                scr_ids = dpool.tile([M], f32, tag="scr_ids")
                nc.sync.dma_start(
                    scr_ids[:].rearrange("(f p) -> p f", p=128), vids[:])
                scr_g = dpool.tile([M], f32, tag="scr_g")
                nc.sync.dma_start(
                    scr_g[:].rearrange("(f p) -> p f", p=128), vg[:])
                vw = router.tile([16, 128], f32)
                nc.sync.dma_start(
                    vw[:], scr_ids[:].rearrange("(f p) -> p f", p=16))
                gw = router.tile([16, 128], f32)
                nc.sync.dma_start(
                    gw[:], scr_g[:].rearrange("(f p) -> p f", p=16))

                idw = router.tile([16, CW], f32)
                cnt_u = router.tile([1, 1], u32)
                nc.gpsimd.sparse_gather(idw[:], vw[:], num_found=cnt_u[:])
                gwc = router.tile([16, CW], f32)
                cnt_u2 = router.tile([1, 1], u32)
                nc.gpsimd.sparse_gather(gwc[:], gw[:], num_found=cnt_u2[:])

                cnt_f = router.tile([1, 1], f32)
                nc.vector.tensor_copy(cnt_f[:], cnt_u[:])
                nc.sync.dma_start(cnt_out[:, :], cnt_f[:])
                nc.sync.dma_start(ids_out[:, :], idw[:])

                cnt_ps = ps_ph1.tile([16, 1], f32, tag="bc")
                nc.tensor.matmul(cnt_ps[:], ones_col[:, 0:16], cnt_f[:])
                cnt_b = router.tile([16, 1], f32)
                nc.vector.tensor_copy(cnt_b[:], cnt_ps[:])

                iota_s = router.tile([16, CW], f32)
                nc.gpsimd.iota(iota_s[:], pattern=[[16, CW]], base=0,
                               channel_multiplier=1,
                               allow_small_or_imprecise_dtypes=True)
                valid = router.tile([16, CW], mybir.dt.uint8)
                nc.vector.tensor_scalar(out=valid[:], in0=iota_s[:],
                                        scalar1=cnt_b[:], scalar2=None,
                                        op0=OP.is_lt)
                zeros16 = router.tile([16, CW], f32)
                nc.vector.memset(zeros16[:], 0.0)
                ids_cl = router.tile([16, CW], f32)
                nc.vector.select(ids_cl[:], valid[:], idw[:], zeros16[:])
                g_cl = router.tile([16, CW], f32)
                nc.vector.select(g_cl[:], valid[:], gwc[:], zeros16[:])
                ids_i16 = router.tile([16, CW], i16)
                nc.vector.tensor_copy(ids_i16[:], ids_cl[:])
                ids_rep = router.tile([128, CW], i16)
                for k in range(8):
                    nc.sync.dma_start(ids_rep[16 * k:16 * (k + 1), :], ids_i16[:])

                # gates wrapped [16, CW] -> slot-partition [128, NCT]
                scr_gs = dpool.tile([CAP], f32, tag="scr_gs")
                nc.sync.dma_start(
                    scr_gs[:].rearrange("(f p) -> p f", p=16), g_cl[:])
                g_slot = router.tile([128, NCT], f32)
                nc.sync.dma_start(
                    g_slot[:], scr_gs[:].rearrange("(t p) -> p t", p=128))

            # ---- gather routed tokens (transpose): xg[p, c, s] ----
            xg = xpool.tile([128, ND, CAP], bf16)
            nc.gpsimd.dma_gather(
                out_ap=xg[:], in_ap=xbf_dram[:, :], idxs_ap=ids_rep[:],
                num_idxs=CAP, num_idxs_reg=CAP, elem_size=D, transpose=True,
            )

            nc.sync.dma_start(dbg_xg[:, :], xg[:].rearrange("p c s -> p (c s)"))
            nc.sync.dma_start(dbg_gslot[:, :], g_slot[:])

            # ---- expert compute ----
            with (
                tc.tile_pool(name="hbuf", bufs=1) as hbuf,
                tc.tile_pool(name="ybuf", bufs=2) as ybuf,
                tc.tile_pool(name="ps_gu", bufs=2, space="PSUM") as ps_gu,
                tc.tile_pool(name="ps_y", bufs=2, space="PSUM") as ps_y,
            ):
                # routed expert on gathered capacity batch
                h_sb = hbuf.tile([128, NPAIR, CAP], bf16, tag="h")
                for pair in range(NPAIR):
                    g_ps = ps_gu.tile([128, CAP], f32, tag="g")
                    u_ps = ps_gu.tile([128, CAP], f32, tag="u")
                    for c in range(ND):
                        nc.tensor.matmul(
                            g_ps[:],
                            w1m_sb[:, c, (2 * pair) * 128:(2 * pair + 1) * 128],
                            xg[:, c, :],
                            start=(c == 0), stop=(c == ND - 1))
                        nc.tensor.matmul(
                            u_ps[:],
                            w1m_sb[:, c, (2 * pair + 1) * 128:(2 * pair + 2) * 128],
                            xg[:, c, :],
                            start=(c == 0), stop=(c == ND - 1))
                    sg = hbuf.tile([128, CAP], f32, tag="sg", bufs=2)
                    nc.scalar.activation(sg[:], g_ps[:], AF.Silu)
                    nc.vector.tensor_tensor(out=h_sb[:, pair, :], in0=sg[:],
                                            in1=u_ps[:], op=OP.mult)
                for tl in range(NCT):
                    y_sb = ybuf.tile([128, D], f32, tag="yout")
                    for dh in range(2):
                        y_ps = ps_y.tile([128, 512], f32, tag="yps")
                        for hc in range(NPAIR):
                            nc.tensor.matmul(
                                y_ps[:],
                                h_sb[:, hc, tl * 128:(tl + 1) * 128],
                                w2m_sb[:, hc, dh * 512:(dh + 1) * 512],
                                start=(hc == 0), stop=(hc == NPAIR - 1))
                        nc.vector.tensor_scalar(
                            out=y_sb[:, dh * 512:(dh + 1) * 512], in0=y_ps[:],
                            scalar1=g_slot[:, tl:tl + 1], scalar2=None,
                            op0=OP.mult)
                    nc.sync.dma_start(yrt_t[:, tl, :], y_sb[:])

                # shared expert on the host-assigned 256-token slice
                hs_sb = hbuf.tile([128, NPAIR, SSH], bf16, tag="hs")
                for pair in range(NPAIR):
                    g_ps = ps_gu.tile([128, SSH], f32, tag="g")
                    u_ps = ps_gu.tile([128, SSH], f32, tag="u")
                    for c in range(ND):
                        nc.tensor.matmul(
                            g_ps[:],
                            w1s_sb[:, c, (2 * pair) * 128:(2 * pair + 1) * 128],
                            xsh_b[:, c, :],
                            start=(c == 0), stop=(c == ND - 1))
                        nc.tensor.matmul(
                            u_ps[:],
                            w1s_sb[:, c, (2 * pair + 1) * 128:(2 * pair + 2) * 128],
                            xsh_b[:, c, :],
                            start=(c == 0), stop=(c == ND - 1))
                    sg = hbuf.tile([128, SSH], f32, tag="sgs", bufs=2)
                    nc.scalar.activation(sg[:], g_ps[:], AF.Silu)
                    nc.vector.tensor_tensor(out=hs_sb[:, pair, :], in0=sg[:],
                                            in1=u_ps[:], op=OP.mult)
                for tl in range(NST):
                    y_sb = ybuf.tile([128, D], f32, tag="yout")
                    for dh in range(2):
                        y_ps = ps_y.tile([128, 512], f32, tag="yps")
                        for hc in range(NPAIR):
                            nc.tensor.matmul(
                                y_ps[:],
                                hs_sb[:, hc, tl * 128:(tl + 1) * 128],
                                w2s_sb[:, hc, dh * 512:(dh + 1) * 512],
                                start=(hc == 0), stop=(hc == NPAIR - 1))
                        nc.vector.tensor_copy(
                            y_sb[:, dh * 512:(dh + 1) * 512], y_ps[:])
                    nc.sync.dma_start(ysh_t[:, tl, :], y_sb[:])

    nc.compile()
    return nc


def _get_built():
    global _built
    if _built is None:
        _built = _build()
    return _built


_built_loop = {}


def _get_built_loop(n):
    if n not in _built_loop:
        _built_loop[n] = _build(loop_n=n)
    return _built_loop[n]


def _prep_w1(W1n):
    """interleave W1 columns into (g_i, u_i) 128-col pairs, bf16"""
    w1r = np.empty((D, 2 * H), dtype=np.float32)
    for i in range(NPAIR):
        w1r[:, (2 * i) * 128:(2 * i + 1) * 128] = W1n[:, i * 128:(i + 1) * 128]
        w1r[:, (2 * i + 1) * 128:(2 * i + 2) * 128] = \
            W1n[:, H + i * 128:H + (i + 1) * 128]
    return w1r.astype(ml_dtypes.bfloat16)


def kernel(x_BSD, Wg_DN, Wl1_ND2H, Wl2_NHD, biases_N):
    x = np.asarray(x_BSD, dtype=np.float32).reshape(M, D)
    Wg = np.ascontiguousarray(np.asarray(Wg_DN, dtype=np.float32))
    W1 = np.asarray(Wl1_ND2H, dtype=np.float32)
    W2 = np.asarray(Wl2_NHD, dtype=np.float32)
    biases = np.asarray(biases_N, dtype=np.float32)

    xT = np.ascontiguousarray(x.T)                       # [D, M] f32
    xbf = x.astype(ml_dtypes.bfloat16)                   # [M, D] bf16

    bias_c = np.maximum(biases, -1e30)
    bias128 = np.tile(bias_c, NT)[None, :].astype(np.float32)

    w1s = _prep_w1(W1[N - 1])
    w2s = np.ascontiguousarray(W2[N - 1]).astype(ml_dtypes.bfloat16)

    nc = _get_built()

    in_maps = []
    for core in range(N):
        sel = np.zeros(N, dtype=np.float32)
        if core < N - 1:
            sel[core] = 1.0
        sel128 = np.tile(sel, NT)[None, :]
        in_maps.append({
            "xT": xT,
            "xbf": xbf,
            "xshT": np.ascontiguousarray(xT[:, core * SSH:(core + 1) * SSH]),
            "wg": Wg,
            "w1m": _prep_w1(W1[min(core, N - 2)]),
            "w2m": np.ascontiguousarray(W2[min(core, N - 2)]).astype(ml_dtypes.bfloat16),
            "w1s": w1s,
            "w2s": w2s,
            "bias128": bias128,
            "sel128": np.ascontiguousarray(sel128),
        })

    global _last_in_maps
    _last_in_maps = in_maps

    res = run_bass_kernel_spmd(nc, in_maps, core_ids=list(range(N)))
    global _last_res
    _last_res = res

    out = np.zeros((M, D), dtype=np.float32)
    for core in range(N):
        r = res.results[core]
        cnt = int(r["cnt_out"][0, 0])
        cnt = min(cnt, CAP)
        ids = r["ids_out"].T.ravel()[:cnt].astype(np.int64)  # wrapped -> slot order
        out[ids] += r["y_rt"][:cnt]
        out[core * SSH:(core + 1) * SSH] += r["y_sh"]
    return out.reshape(B, S, D)


# revision 16
# speedup vs baseline: 1359.9535x; 1.3613x over previous
"""MoE (top-1 routed + 1 shared expert) Trainium2 kernel.

Strategy (v2, sparse expert-parallel with on-device dispatch):
  - 8 NeuronCores. Core n (n<7) owns routed expert n; every core also computes
    a 256-token slice of the shared expert (expert 7).
  - Each core: f32 router over all 2048 tokens (logits = x @ Wg, top-1 among
    bias-eligible experts), on-device stream-compaction (gpsimd sparse_gather)
    of the tokens routed to this core's expert, capacity C=384, then a
    gather+transpose DMA pulls just those token rows (bf16) and the expert
    runs dense on the compacted [C] batch. Shared expert runs on a host-sliced
    256-token column block. Outputs are compact: [C, D] routed rows + token
    ids + count + [256, D] shared rows.
  - Host combine: scatter-add of the disjoint routed rows + slice-add of the
    shared rows.

Shapes hardcoded: B=2, S=1024, D=1024, H=1024, N=8, top-1 routed + shared.
"""

import numpy as np
import ml_dtypes

import concourse.bass as bass
import concourse.mybir as mybir
from concourse import bacc
from concourse.tile import TileContext
from concourse.bass_utils import run_bass_kernel_spmd

B, S, D, H, N = 2, 1024, 1024, 1024, 8
M = B * S            # 2048 tokens
NT = M // 128        # 16 token tiles
ND = D // 128        # 8 contraction chunks
NPAIR = H // 128     # 8 (g,u) pairs of 128-wide h tiles
CAP = 384            # routed token capacity per expert
NCT = CAP // 128     # 3 capacity tiles
CW = CAP // 16       # 24 wrapped columns
SSH = M // 8         # 256 shared tokens per core
NST = SSH // 128     # 2 shared token tiles

f32 = mybir.dt.float32
bf16 = mybir.dt.bfloat16
i16 = mybir.dt.int16
u32 = mybir.dt.uint32
AF = mybir.ActivationFunctionType
OP = mybir.AluOpType
AX = mybir.AxisListType

_built = None


def _build(loop_n=None):
    import contextlib

    nc = bacc.Bacc("TRN2", target_bir_lowering=False, debug=False)

    xT = nc.dram_tensor("xT", [D, M], f32, kind="ExternalInput")
    xbf_dram = nc.dram_tensor("xbf", [M, D], bf16, kind="ExternalInput")

    wg = nc.dram_tensor("wg", [D, N], f32, kind="ExternalInput")
    w1m = nc.dram_tensor("w1m", [D, 2 * H], bf16, kind="ExternalInput")
    w2m = nc.dram_tensor("w2m", [H, D], bf16, kind="ExternalInput")
    w1s = nc.dram_tensor("w1s", [D, 256], bf16, kind="ExternalInput")
    w2s = nc.dram_tensor("w2s", [128, D], bf16, kind="ExternalInput")
    bias128 = nc.dram_tensor("bias128", [1, 128], f32, kind="ExternalInput")
    sel128 = nc.dram_tensor("sel128", [1, 128], f32, kind="ExternalInput")

    y_rt = nc.dram_tensor("y_rt", [CAP, D], f32, kind="ExternalOutput")
    y_sh = nc.dram_tensor("y_sh", [M, D], bf16, kind="ExternalOutput")
    ids_out = nc.dram_tensor("ids_out", [16, CW], f32, kind="ExternalOutput")
    dbg_xg = nc.dram_tensor("dbg_xg", [128, ND * CAP], bf16, kind="ExternalOutput")
    dbg_gslot = nc.dram_tensor("dbg_gslot", [128, NCT], f32, kind="ExternalOutput")
    cnt_out = nc.dram_tensor("cnt_out", [1, 1], f32, kind="ExternalOutput")

    xT_t = xT[:, :].rearrange("(c p) m -> p c m", p=128)       # [128, ND, M]
    wg_t = wg[:, :].rearrange("(c p) n -> p c n", p=128)
    w1m_t = w1m[:, :].rearrange("(c p) h -> p c h", p=128)
    w2m_t = w2m[:, :].rearrange("(c p) d -> p c d", p=128)
    w1s_t = w1s[:, :].rearrange("(c p) h -> p c h", p=128)
    w2s_t = w2s[:, :].rearrange("(c p) d -> p c d", p=128)
    yrt_t = y_rt[:, :].rearrange("(t p) d -> p t d", p=128)    # [128, NCT, D]
    ysh_t = y_sh[:, :].rearrange("(t p) d -> p t d", p=128)    # [128, NT, D]

    with TileContext(nc) as tc:
        loop_ctx = tc.For_i(0, loop_n, 1) if loop_n else contextlib.nullcontext()
        with (
            loop_ctx,
            tc.tile_pool(name="consts", bufs=1) as consts,
            tc.tile_pool(name="weights", bufs=1) as wpool,
            tc.tile_pool(name="xpool", bufs=1) as xpool,
            tc.tile_pool(name="router", bufs=1) as router,
            tc.tile_pool(name="dram", bufs=1, space="DRAM") as dpool,
        ):
            # ---- constants / weights ----
            wg_sb = consts.tile([128, ND, N], f32)
            nc.sync.dma_start(wg_sb[:], wg_t)
            bias_row = consts.tile([1, 128], f32)
            nc.sync.dma_start(bias_row[:], bias128[:, :])
            sel_row = consts.tile([1, 128], f32)
            nc.sync.dma_start(sel_row[:], sel128[:, :])
            ones_col = consts.tile([1, 128], f32)
            nc.vector.memset(ones_col[:], 1.0)

            w1s_sb = wpool.tile([128, ND, 256], bf16, tag="w1s")
            nc.sync.dma_start(w1s_sb[:], w1s_t)
            w2s_sb = wpool.tile([128, D], bf16, tag="w2s")
            nc.sync.dma_start(w2s_sb[:], w2s[:, :].rearrange("(c p) d -> p (c d)", p=128))
            w1m_sb = wpool.tile([128, ND, 2 * H], bf16, tag="w1m")
            nc.sync.dma_start(w1m_sb[:], w1m_t)
            w2m_sb = wpool.tile([128, NPAIR, D], bf16, tag="w2m")
            nc.sync.dma_start(w2m_sb[:], w2m_t)

            # ---- load xT (f32, resident) and shared-slice xT ----
            xtf = xpool.tile([128, ND, M], f32)
            for c in range(ND):
                nc.sync.dma_start(xtf[:, c, :], xT_t[:, c, :])
            xsh_b = xpool.tile([128, ND, M], bf16)
            for c in range(ND):
                nc.vector.tensor_copy(xsh_b[:, c, :], xtf[:, c, :])

            with tc.tile_pool(name="ps_ph1", bufs=2, space="PSUM") as ps_ph1:
                # broadcast rows across partitions via ones-matmul (K=1)
                bias_ps = ps_ph1.tile([128, 128], f32, tag="bc")
                nc.tensor.matmul(bias_ps[:], ones_col[:], bias_row[:])
                bias_rep = consts.tile([128, 128], f32)
                nc.vector.tensor_copy(bias_rep[:], bias_ps[:])

                sel_ps = ps_ph1.tile([128, 128], f32, tag="bc")
                nc.tensor.matmul(sel_ps[:], ones_col[:], sel_row[:])
                sel_rep = consts.tile([128, 128], f32)
                nc.vector.tensor_copy(sel_rep[:], sel_ps[:])

                # router matmuls: one accumulation group at a time, ping-pong
                logits_sb = router.tile([128, NT, N], f32)
                for tt in range(NT):
                    lg_ps = ps_ph1.tile([128, N], f32, tag="lg")
                    for c in range(ND):
                        nc.tensor.matmul(
                            lg_ps[:],
                            xtf[:, c, tt * 128:(tt + 1) * 128],
                            wg_sb[:, c, :],
                            start=(c == 0), stop=(c == ND - 1),
                        )
                    nc.vector.tensor_copy(logits_sb[:, tt, :], lg_ps[:])

                # ---- router epilogue: mask + gate in [tok%128, tok//128] ----
                scores = router.tile([128, NT, N], f32)
                nc.scalar.activation(scores[:].rearrange("p t n -> p (t n)"),
                                     logits_sb[:].rearrange("p t n -> p (t n)"),
                                     AF.Sigmoid)
                tsel = router.tile([128, NT, N], f32)
                nc.vector.tensor_tensor(
                    out=tsel[:].rearrange("p t n -> p (t n)"),
                    in0=logits_sb[:].rearrange("p t n -> p (t n)"),
                    in1=bias_rep[:], op=OP.add)
                tmax = router.tile([128, NT], f32)
                nc.vector.tensor_reduce(tmax[:], tsel[:], axis=AX.X, op=OP.max)
                oh = router.tile([128, NT, N], f32)
                for tt in range(NT):
                    nc.vector.tensor_scalar(
                        out=oh[:, tt, :], in0=tsel[:, tt, :],
                        scalar1=tmax[:, tt:tt + 1], scalar2=None,
                        op0=OP.is_equal)
                ohs = router.tile([128, NT, N], f32)
                nc.vector.tensor_tensor(
                    out=ohs[:].rearrange("p t n -> p (t n)"),
                    in0=oh[:].rearrange("p t n -> p (t n)"),
                    in1=scores[:].rearrange("p t n -> p (t n)"), op=OP.mult)
                s_sel = router.tile([128, NT], f32)
                nc.vector.tensor_reduce(s_sel[:], ohs[:], axis=AX.X, op=OP.add)
                den = router.tile([128, NT], f32)
                nc.vector.tensor_scalar(out=den[:], in0=s_sel[:], scalar1=1e-9,
                                        scalar2=None, op0=OP.add)
                den_r = router.tile([128, NT], f32)
                nc.vector.reciprocal(den_r[:], den[:])
                wgt = router.tile([128, NT], f32)
                nc.vector.tensor_tensor(out=wgt[:], in0=s_sel[:], in1=den_r[:],
                                        op=OP.mult)
                ohm = router.tile([128, NT, N], f32)
                nc.vector.tensor_tensor(
                    out=ohm[:].rearrange("p t n -> p (t n)"),
                    in0=oh[:].rearrange("p t n -> p (t n)"),
                    in1=sel_rep[:], op=OP.mult)
                msk = router.tile([128, NT], f32)
                nc.vector.tensor_reduce(msk[:], ohm[:], axis=AX.X, op=OP.add)
                gate = router.tile([128, NT], f32)
                nc.vector.tensor_tensor(out=gate[:], in0=msk[:], in1=wgt[:],
                                        op=OP.mult)

                # ---- compaction: ids/gates of selected tokens ----
                iota_t = router.tile([128, NT], f32)
                nc.gpsimd.iota(iota_t[:], pattern=[[128, NT]], base=1,
                               channel_multiplier=1,
                               allow_small_or_imprecise_dtypes=True)
                vids = router.tile([128, NT], f32)
                # msk*(m+1) - 1  ->  m if selected else -1
                nc.vector.tensor_tensor(out=vids[:], in0=msk[:], in1=iota_t[:],
                                        op=OP.mult)
                nc.vector.tensor_scalar(out=vids[:], in0=vids[:], scalar1=-1.0,
                                        scalar2=None, op0=OP.add)
                gp1 = router.tile([128, NT], f32)
                nc.vector.tensor_scalar(out=gp1[:], in0=gate[:], scalar1=1.0,
                                        scalar2=None, op0=OP.add)
                vg = router.tile([128, NT], f32)
                nc.vector.tensor_tensor(out=vg[:], in0=msk[:], in1=gp1[:],
                                        op=OP.mult)
                nc.vector.tensor_scalar(out=vg[:], in0=vg[:], scalar1=-1.0,
                                        scalar2=None, op0=OP.add)

                # bounce [128, NT] (tok-partition) -> [16, 128] (wrapped)
                scr_ids = dpool.tile([M], f32, tag="scr_ids")
                nc.sync.dma_start(
                    scr_ids[:].rearrange("(f p) -> p f", p=128), vids[:])
                scr_g = dpool.tile([M], f32, tag="scr_g")
                nc.sync.dma_start(
                    scr_g[:].rearrange("(f p) -> p f", p=128), vg[:])
                vw = router.tile([16, 128], f32)
                nc.sync.dma_start(
                    vw[:], scr_ids[:].rearrange("(f p) -> p f", p=16))
                gw = router.tile([16, 128], f32)
                nc.sync.dma_start(
                    gw[:], scr_g[:].rearrange("(f p) -> p f", p=16))

                idw = router.tile([16, CW], f32)
                cnt_u = router.tile([1, 1], u32)
                nc.gpsimd.sparse_gather(idw[:], vw[:], num_found=cnt_u[:])
                gwc = router.tile([16, CW], f32)
                cnt_u2 = router.tile([1, 1], u32)
                nc.gpsimd.sparse_gather(gwc[:], gw[:], num_found=cnt_u2[:])

                cnt_f = router.tile([1, 1], f32)
                nc.vector.tensor_copy(cnt_f[:], cnt_u[:])
                nc.sync.dma_start(cnt_out[:, :], cnt_f[:])
                nc.sync.dma_start(ids_out[:, :], idw[:])

                cnt_ps = ps_ph1.tile([16, 1], f32, tag="bc")
                nc.tensor.matmul(cnt_ps[:], ones_col[:, 0:16], cnt_f[:])
                cnt_b = router.tile([16, 1], f32)
                nc.vector.tensor_copy(cnt_b[:], cnt_ps[:])

                iota_s = router.tile([16, CW], f32)
                nc.gpsimd.iota(iota_s[:], pattern=[[16, CW]], base=0,
                               channel_multiplier=1,
                               allow_small_or_imprecise_dtypes=True)
                valid = router.tile([16, CW], mybir.dt.uint8)
                nc.vector.tensor_scalar(out=valid[:], in0=iota_s[:],
                                        scalar1=cnt_b[:], scalar2=None,
                                        op0=OP.is_lt)
                zeros16 = router.tile([16, CW], f32)
                nc.vector.memset(zeros16[:], 0.0)
                ids_cl = router.tile([16, CW], f32)
                nc.vector.select(ids_cl[:], valid[:], idw[:], zeros16[:])
                g_cl = router.tile([16, CW], f32)
                nc.vector.select(g_cl[:], valid[:], gwc[:], zeros16[:])
                ids_i16 = router.tile([16, CW], i16)
                nc.vector.tensor_copy(ids_i16[:], ids_cl[:])
                ids_rep = router.tile([128, CW], i16)
                for k in range(8):
                    nc.sync.dma_start(ids_rep[16 * k:16 * (k + 1), :], ids_i16[:])

                # gates wrapped [16, CW] -> slot-partition [128, NCT]
                scr_gs = dpool.tile([CAP], f32, tag="scr_gs")
                nc.sync.dma_start(
                    scr_gs[:].rearrange("(f p) -> p f", p=16), g_cl[:])
                g_slot = router.tile([128, NCT], f32)
                nc.sync.dma_start(
                    g_slot[:], scr_gs[:].rearrange("(t p) -> p t", p=128))

            # ---- gather routed tokens (transpose): xg[p, c, s] ----
            xg = xpool.tile([128, ND, CAP], bf16)
            nc.gpsimd.dma_gather(
                out_ap=xg[:], in_ap=xbf_dram[:, :], idxs_ap=ids_rep[:],
                num_idxs=CAP, num_idxs_reg=CAP, elem_size=D, transpose=True,
            )

            nc.sync.dma_start(dbg_xg[:, :], xg[:].rearrange("p c s -> p (c s)"))
            nc.sync.dma_start(dbg_gslot[:, :], g_slot[:])

            # ---- expert compute ----
            with (
                tc.tile_pool(name="hbuf", bufs=1) as hbuf,
                tc.tile_pool(name="ybuf", bufs=2) as ybuf,
                tc.tile_pool(name="ps_gu", bufs=2, space="PSUM") as ps_gu,
                tc.tile_pool(name="ps_y", bufs=2, space="PSUM") as ps_y,
            ):
                # routed expert on gathered capacity batch
                h_sb = hbuf.tile([128, NPAIR, CAP], bf16, tag="h")
                for pair in range(NPAIR):
                    g_ps = ps_gu.tile([128, CAP], f32, tag="g")
                    u_ps = ps_gu.tile([128, CAP], f32, tag="u")
                    for c in range(ND):
                        nc.tensor.matmul(
                            g_ps[:],
                            w1m_sb[:, c, (2 * pair) * 128:(2 * pair + 1) * 128],
                            xg[:, c, :],
                            start=(c == 0), stop=(c == ND - 1))
                        nc.tensor.matmul(
                            u_ps[:],
                            w1m_sb[:, c, (2 * pair + 1) * 128:(2 * pair + 2) * 128],
                            xg[:, c, :],
                            start=(c == 0), stop=(c == ND - 1))
                    sg = hbuf.tile([128, CAP], f32, tag="sg", bufs=2)
                    nc.scalar.activation(sg[:], g_ps[:], AF.Silu)
                    nc.vector.tensor_tensor(out=h_sb[:, pair, :], in0=sg[:],
                                            in1=u_ps[:], op=OP.mult)
                for tl in range(NCT):
                    y_sb = ybuf.tile([128, D], f32, tag="yout")
                    for dh in range(2):
                        y_ps = ps_y.tile([128, 512], f32, tag="yps")
                        for hc in range(NPAIR):
                            nc.tensor.matmul(
                                y_ps[:],
                                h_sb[:, hc, tl * 128:(tl + 1) * 128],
                                w2m_sb[:, hc, dh * 512:(dh + 1) * 512],
                                start=(hc == 0), stop=(hc == NPAIR - 1))
                        nc.vector.tensor_scalar(
                            out=y_sb[:, dh * 512:(dh + 1) * 512], in0=y_ps[:],
                            scalar1=g_slot[:, tl:tl + 1], scalar2=None,
                            op0=OP.mult)
                    nc.sync.dma_start(yrt_t[:, tl, :], y_sb[:])

                # shared expert: this core's 128-wide H slice over ALL tokens
                hs_sb = hbuf.tile([128, M], bf16, tag="hs")
                for tkc in range(4):
                    tok = slice(tkc * 512, (tkc + 1) * 512)
                    g_ps = ps_gu.tile([128, 512], f32, tag="g")
                    u_ps = ps_gu.tile([128, 512], f32, tag="u")
                    for c in range(ND):
                        nc.tensor.matmul(
                            g_ps[:], w1s_sb[:, c, 0:128], xsh_b[:, c, tok],
                            start=(c == 0), stop=(c == ND - 1))
                        nc.tensor.matmul(
                            u_ps[:], w1s_sb[:, c, 128:256], xsh_b[:, c, tok],
                            start=(c == 0), stop=(c == ND - 1))
                    sg = hbuf.tile([128, 512], f32, tag="sgs", bufs=2)
                    nc.scalar.activation(sg[:], g_ps[:], AF.Silu)
                    nc.vector.tensor_tensor(out=hs_sb[:, tok], in0=sg[:],
                                            in1=u_ps[:], op=OP.mult)
                for tl in range(NT):
                    y_sb = ybuf.tile([128, D], bf16, tag="ysout")
                    for dh in range(2):
                        y_ps = ps_y.tile([128, 512], f32, tag="yps")
                        nc.tensor.matmul(
                            y_ps[:],
                            hs_sb[:, tl * 128:(tl + 1) * 128],
                            w2s_sb[:, dh * 512:(dh + 1) * 512],
                            start=True, stop=True)
                        nc.vector.tensor_copy(
                            y_sb[:, dh * 512:(dh + 1) * 512], y_ps[:])
                    nc.sync.dma_start(ysh_t[:, tl, :], y_sb[:])

    nc.compile()
    return nc


def _get_built():
    global _built
    if _built is None:
        _built = _build()
    return _built


_built_loop = {}


def _get_built_loop(n):
    if n not in _built_loop:
        _built_loop[n] = _build(loop_n=n)
    return _built_loop[n]


def _prep_w1(W1n):
    """interleave W1 columns into (g_i, u_i) 128-col pairs, bf16"""
    w1r = np.empty((D, 2 * H), dtype=np.float32)
    for i in range(NPAIR):
        w1r[:, (2 * i) * 128:(2 * i + 1) * 128] = W1n[:, i * 128:(i + 1) * 128]
        w1r[:, (2 * i + 1) * 128:(2 * i + 2) * 128] = \
            W1n[:, H + i * 128:H + (i + 1) * 128]
    return w1r.astype(ml_dtypes.bfloat16)


def kernel(x_BSD, Wg_DN, Wl1_ND2H, Wl2_NHD, biases_N):
    x = np.asarray(x_BSD, dtype=np.float32).reshape(M, D)
    Wg = np.ascontiguousarray(np.asarray(Wg_DN, dtype=np.float32))
    W1 = np.asarray(Wl1_ND2H, dtype=np.float32)
    W2 = np.asarray(Wl2_NHD, dtype=np.float32)
    biases = np.asarray(biases_N, dtype=np.float32)

    xT = np.ascontiguousarray(x.T)                       # [D, M] f32
    xbf = x.astype(ml_dtypes.bfloat16)                   # [M, D] bf16

    bias_c = np.maximum(biases, -1e30)
    bias128 = np.tile(bias_c, NT)[None, :].astype(np.float32)


    nc = _get_built()

    in_maps = []
    for core in range(N):
        sel = np.zeros(N, dtype=np.float32)
        if core < N - 1:
            sel[core] = 1.0
        sel128 = np.tile(sel, NT)[None, :]
        hlo = core * 128
        w1s_c = np.concatenate(
            [W1[N - 1][:, hlo:hlo + 128], W1[N - 1][:, H + hlo:H + hlo + 128]],
            axis=1)
        in_maps.append({
            "xT": xT,
            "xbf": xbf,
            "wg": Wg,
            "w1m": _prep_w1(W1[min(core, N - 2)]),
            "w2m": np.ascontiguousarray(W2[min(core, N - 2)]).astype(ml_dtypes.bfloat16),
            "w1s": np.ascontiguousarray(w1s_c).astype(ml_dtypes.bfloat16),
            "w2s": np.ascontiguousarray(W2[N - 1][hlo:hlo + 128, :]).astype(ml_dtypes.bfloat16),
            "bias128": bias128,
            "sel128": np.ascontiguousarray(sel128),
        })

    global _last_in_maps
    _last_in_maps = in_maps

    res = run_bass_kernel_spmd(nc, in_maps, core_ids=list(range(N)))
    global _last_res
    _last_res = res

    out = np.zeros((M, D), dtype=np.float32)
    for core in range(N):
        r = res.results[core]
        cnt = int(r["cnt_out"][0, 0])
        cnt = min(cnt, CAP)
        ids = r["ids_out"].T.ravel()[:cnt].astype(np.int64)  # wrapped -> slot order
        out[ids] += r["y_rt"][:cnt]
        out += r["y_sh"].astype(np.float32)
    return out.reshape(B, S, D)
